# revision 1
# baseline (speedup 1.0000x reference)
"""DescrptSeA descriptor kernel for 8 Trainium2 NeuronCores.

Data-parallel sharding per the problem's sharding hint: the nloc (local atom)
axis is split into 8 equal shards, one per core. The neighbor-list gather is
performed host-side (the neuron compiler's codegen currently asserts on
indirect_load for this access pattern); each core then computes the smoothed
environment matrix, runs the 3-layer embedding net (1->25->50->100, tanh,
resnet-doubling skips) over its (atom, neighbor) points, and contracts to the
[M*AXIS] descriptor on device. Shard outputs are concatenated on the host into
the full [nf, nloc, 1600] float32 output.
"""

import numpy as np
import jax
import jax.numpy as jnp

NF, NLOC, NALL = 2, 4096, 8192
NTYPES = 2
SEL = [46, 92]
NNEI = sum(SEL)
SEC = [0, 46, 138]
NEURON = [25, 50, 100]
AXIS = 16
RCUT, RCUT_SMTH = 6.0, 0.5
PROT = 1e-6

NCORES = 8
SHARD = NLOC // NCORES  # 512 atoms per core


def _smooth_weight(d, rmin, rmax):
    uu = (d - rmin) / (rmax - rmin)
    uu = jnp.clip(uu, 0.0, 1.0)
    return uu * uu * uu * (-6.0 * uu * uu + 15.0 * uu - 10.0) + 1.0


def _shard_fn(coord_r, centers, mask, nscale, nshift,
              w0, b0, w1, b1, w2, b2):
    # coord_r [nf, shard, nnei, 3]; centers [nf, shard, 3]
    # mask [nf, shard, nnei] f32; nscale/nshift [nf, shard, nnei, 4] f32
    nf, nloc, nnei, _ = coord_r.shape
    diff = coord_r - centers[:, :, None, :]
    length = jnp.sqrt(jnp.sum(diff * diff, axis=-1, keepdims=True))
    m = mask[..., None]
    length = length * m + (1.0 - m)  # padding entries -> length 1.0
    t0 = 1.0 / (length + PROT)
    t1 = diff / ((length + PROT) ** 2)
    w = _smooth_weight(length, RCUT_SMTH, RCUT) * m
    env = jnp.concatenate([t0, t1], axis=-1) * w
    dm = env * nscale + nshift

    dm = dm.reshape(nf * nloc, nnei, 4)
    M = w2.shape[-1]
    xyz = jnp.zeros((nf * nloc, 4, M), dm.dtype)
    for t in range(NTYPES):
        rr = dm[:, SEC[t] : SEC[t + 1], :]
        x = rr[:, :, :1]
        for W, b in ((w0[t], b0[t]), (w1[t], b1[t]), (w2[t], b2[t])):
            y = jnp.tanh(x @ W + b)
            if W.shape[-1] == 2 * x.shape[-1]:
                y = y + jnp.concatenate([x, x], axis=-1)
            elif W.shape[-1] == x.shape[-1]:
                y = y + x
            x = y
        xyz = xyz + jnp.einsum("nsc,nsm->ncm", rr, x)
    xyz = xyz / NNEI
    res = jnp.einsum("ncm,nca->nma", xyz, xyz[:, :, :AXIS])
    return res.reshape(nf, nloc, M * AXIS)


_pmapped = None


def _get_pmapped():
    global _pmapped
    if _pmapped is None:
        _pmapped = jax.pmap(_shard_fn, devices=jax.devices()[:NCORES])
    return _pmapped


def kernel(nlist, extended_coord, extended_atype, mean, stddev,
           w0, b0, w1, b1, w2, b2):
    nlist = np.asarray(nlist)
    coord = np.asarray(extended_coord, dtype=np.float32)  # [nf, nall, 3]
    atype = np.asarray(extended_atype)

    mask = (nlist >= 0)
    nl = np.where(mask, nlist, 0).astype(np.int64)  # [nf, nloc, nnei]

    # host-side neighbor gather (index marshalling for the device kernel)
    fidx = np.arange(NF)[:, None, None]
    coord_r = coord[fidx, nl, :]  # [nf, nloc, nnei, 3]
    centers = coord[:, :NLOC, :]  # [nf, nloc, 3]

    # shard the nloc axis across the 8 cores
    def sh(x, extra):  # [nf, nloc, ...] -> [8, nf, shard, ...]
        return np.ascontiguousarray(
            x.reshape((NF, NCORES, SHARD) + extra).transpose((1, 0, 2) + tuple(
                3 + i for i in range(len(extra))))
        )

    coord_r_sh = sh(coord_r, (NNEI, 3)).astype(np.float32)
    centers_sh = sh(centers, (3,)).astype(np.float32)
    mask_sh = sh(mask.astype(np.float32), (NNEI,))
    atype_loc = atype[:, :NLOC].astype(np.int64)
    nscale = (1.0 / np.asarray(stddev, np.float32))[atype_loc]  # [nf,nloc,nnei,4]
    nshift = (-np.asarray(mean, np.float32) / np.asarray(stddev, np.float32))[atype_loc]
    nscale_sh = sh(nscale, (NNEI, 4)).astype(np.float32)
    nshift_sh = sh(nshift, (NNEI, 4)).astype(np.float32)

    def rep(x):
        x = np.asarray(x, dtype=np.float32)
        return np.broadcast_to(x, (NCORES,) + x.shape).copy()

    out = _get_pmapped()(
        coord_r_sh, centers_sh, mask_sh, nscale_sh, nshift_sh,
        rep(w0), rep(b0), rep(w1), rep(b1), rep(w2), rep(b2),
    )  # [8, nf, shard, M*AXIS]

    out = np.asarray(out)
    full = out.transpose(1, 0, 2, 3).reshape(NF, NLOC, NEURON[-1] * AXIS)
    return np.ascontiguousarray(full.astype(np.float32))



# revision 2
# speedup vs baseline: 2.8428x; 2.8428x over previous
"""DescrptSeA descriptor kernel for 8 Trainium2 NeuronCores.

Data-parallel over the nloc axis (512 atoms/core). The neighbor gather runs
on-device via a flat jnp.take (the take_along_axis form trips a neuron
compiler assert, the flat form compiles). Host work is limited to index
marshalling: baking the frame offset into nlist and slicing per-core shards.
Only ~6 MB is shipped in (int32 nlist shards + replicated coords + tiny
tables) and ~13 MB out (bf16), instead of ~107 MB for the fully host-gathered
variant. The embedding net (1->25->50->100, tanh, resnet-doubling skips), the
env-mat normalization (type-select via where, no indexed gather), and the
final contraction all run on device; output is cast back to f32 on host.
"""

import numpy as np
import jax
import jax.numpy as jnp

NF, NLOC, NALL = 2, 4096, 8192
NTYPES = 2
SEL = [46, 92]
NNEI = sum(SEL)
SEC = [0, 46, 138]
NEURON = [25, 50, 100]
AXIS = 16
RCUT, RCUT_SMTH = 6.0, 0.5
PROT = 1e-6

NCORES = 8
SHARD = NLOC // NCORES  # 512 atoms per core


def _smooth_weight(d, rmin, rmax):
    uu = (d - rmin) / (rmax - rmin)
    uu = jnp.clip(uu, 0.0, 1.0)
    return uu * uu * uu * (-6.0 * uu * uu + 15.0 * uu - 10.0) + 1.0


def _shard_fn(nl_abs, mask_u8, coord_all, centers, atype_loc, mean, std,
              w0, b0, w1, b1, w2, b2):
    # nl_abs [nf, shard, nnei] int32 with frame offset baked in
    # coord_all [nf*nall, 3] f32; centers [nf, shard, 3]
    # atype_loc [nf, shard] int32; mean/std [ntypes, nnei, 4]
    nf, nloc, nnei = nl_abs.shape
    mask = mask_u8.astype(jnp.float32)

    coord_r = jnp.take(coord_all, nl_abs.reshape(-1), axis=0)
    coord_r = coord_r.reshape(nf, nloc, nnei, 3)
    diff = coord_r - centers[:, :, None, :]
    length = jnp.sqrt(jnp.sum(diff * diff, axis=-1, keepdims=True))
    m = mask[..., None]
    length = length * m + (1.0 - m)
    t0 = 1.0 / (length + PROT)
    t1 = diff / ((length + PROT) ** 2)
    w = _smooth_weight(length, RCUT_SMTH, RCUT) * m
    env = jnp.concatenate([t0, t1], axis=-1) * w  # [nf, shard, nnei, 4]

    is0 = (atype_loc == 0)[:, :, None, None]
    mean_sel = jnp.where(is0, mean[0], mean[1])
    std_sel = jnp.where(is0, std[0], std[1])
    dm = (env - mean_sel) / std_sel

    dm = dm.reshape(nf * nloc, nnei, 4)
    M = w2.shape[-1]
    xyz = jnp.zeros((nf * nloc, 4, M), dm.dtype)
    for t in range(NTYPES):
        rr = dm[:, SEC[t] : SEC[t + 1], :]
        x = rr[:, :, :1]
        for W, b in ((w0[t], b0[t]), (w1[t], b1[t]), (w2[t], b2[t])):
            y = jnp.tanh(x @ W + b)
            if W.shape[-1] == 2 * x.shape[-1]:
                y = y + jnp.concatenate([x, x], axis=-1)
            elif W.shape[-1] == x.shape[-1]:
                y = y + x
            x = y
        xyz = xyz + jnp.einsum("nsc,nsm->ncm", rr, x)
    xyz = xyz / NNEI
    res = jnp.einsum("ncm,nca->nma", xyz, xyz[:, :, :AXIS])
    return res.reshape(nf, nloc, M * AXIS).astype(jnp.bfloat16)


_pmapped = None


def _get_pmapped():
    global _pmapped
    if _pmapped is None:
        _pmapped = jax.pmap(_shard_fn, devices=jax.devices()[:NCORES])
    return _pmapped


def kernel(nlist, extended_coord, extended_atype, mean, stddev,
           w0, b0, w1, b1, w2, b2):
    nlist = np.asarray(nlist)
    coord = np.asarray(extended_coord, dtype=np.float32)  # [nf, nall, 3]
    atype = np.asarray(extended_atype)

    mask = nlist >= 0
    # bake the frame offset into the indices for a single flat gather
    frame_off = (np.arange(NF, dtype=np.int32) * NALL)[:, None, None]
    nl_abs = (np.where(mask, nlist, 0).astype(np.int32) + frame_off)

    # shard the nloc axis across the 8 cores: [nf, nloc, ...] -> [8, nf, 512, ...]
    nl_sh = np.ascontiguousarray(
        nl_abs.reshape(NF, NCORES, SHARD, NNEI).transpose(1, 0, 2, 3))
    mask_sh = np.ascontiguousarray(
        mask.reshape(NF, NCORES, SHARD, NNEI).transpose(1, 0, 2, 3)
    ).astype(np.uint8)
    centers_sh = np.ascontiguousarray(
        coord[:, :NLOC].reshape(NF, NCORES, SHARD, 3).transpose(1, 0, 2, 3))
    atype_sh = np.ascontiguousarray(
        atype[:, :NLOC].astype(np.int32).reshape(NF, NCORES, SHARD)
        .transpose(1, 0, 2))

    coord_all = coord.reshape(NF * NALL, 3)

    def rep(x):
        x = np.asarray(x, dtype=np.float32)
        return np.broadcast_to(x, (NCORES,) + x.shape)

    out = _get_pmapped()(
        nl_sh, mask_sh, rep(coord_all), centers_sh, atype_sh,
        rep(mean), rep(stddev),
        rep(w0), rep(b0), rep(w1), rep(b1), rep(w2), rep(b2),
    )  # [8, nf, shard, M*AXIS] bf16

    out = np.asarray(out).astype(np.float32)
    full = out.transpose(1, 0, 2, 3).reshape(NF, NLOC, NEURON[-1] * AXIS)
    return np.ascontiguousarray(full)


# revision 3
# speedup vs baseline: 6.6339x; 2.3336x over previous
"""DescrptSeA descriptor kernel for 8 Trainium2 NeuronCores.

Data-parallel over the nloc axis (512 atoms/core). The neighbor gather runs
on-device via a flat jnp.take (the take_along_axis form trips a neuron
compiler assert; the flat form compiles). Wire traffic is minimized: in go
int16 neighbor indices with the mask folded into the sign bit (~2.3 MB),
replicated coords (~1.6 MB) and tiny weight tables; out comes only the
rank-4 factor xyz = rr^T @ gg per atom ([nf, 512, 4, 100] bf16, ~6.5 MB)
instead of the full 26-52 MB descriptor. The final res = xyz^T @ xyz[:, :16]
outer product is cheap (52 MFLOP) and runs on host BLAS in fp32.
"""

import numpy as np
import jax
import jax.numpy as jnp

NF, NLOC, NALL = 2, 4096, 8192
NTYPES = 2
SEL = [46, 92]
NNEI = sum(SEL)
SEC = [0, 46, 138]
NEURON = [25, 50, 100]
AXIS = 16
RCUT, RCUT_SMTH = 6.0, 0.5
PROT = 1e-6

NCORES = 8
SHARD = NLOC // NCORES  # 512 atoms per core


def _smooth_weight(d, rmin, rmax):
    uu = (d - rmin) / (rmax - rmin)
    uu = jnp.clip(uu, 0.0, 1.0)
    return uu * uu * uu * (-6.0 * uu * uu + 15.0 * uu - 10.0) + 1.0


def _shard_fn(nl_i16, coord_all, centers, atype_loc, mean, std,
              w0, b0, w1, b1, w2, b2):
    # nl_i16 [nf, shard, nnei] int16: frame-offset index, negative = padded
    # coord_all [nf*nall, 3] f32; centers [nf, shard, 3]
    nf, nloc, nnei = nl_i16.shape
    mask = (nl_i16 >= 0)
    nl = jnp.where(mask, nl_i16, 0).astype(jnp.int32)
    m = mask[..., None].astype(jnp.float32)

    coord_r = jnp.take(coord_all, nl.reshape(-1), axis=0)
    coord_r = coord_r.reshape(nf, nloc, nnei, 3)
    diff = coord_r - centers[:, :, None, :]
    length = jnp.sqrt(jnp.sum(diff * diff, axis=-1, keepdims=True))
    length = length * m + (1.0 - m)
    t0 = 1.0 / (length + PROT)
    t1 = diff / ((length + PROT) ** 2)
    w = _smooth_weight(length, RCUT_SMTH, RCUT) * m
    env = jnp.concatenate([t0, t1], axis=-1) * w  # [nf, shard, nnei, 4]

    is0 = (atype_loc == 0)[:, :, None, None]
    mean_sel = jnp.where(is0, mean[0], mean[1])
    std_sel = jnp.where(is0, std[0], std[1])
    dm = (env - mean_sel) / std_sel

    dm = dm.reshape(nf * nloc, nnei, 4)
    M = w2.shape[-1]
    xyz = jnp.zeros((nf * nloc, 4, M), dm.dtype)
    for t in range(NTYPES):
        rr = dm[:, SEC[t] : SEC[t + 1], :]
        x = rr[:, :, :1]
        for W, b in ((w0[t], b0[t]), (w1[t], b1[t]), (w2[t], b2[t])):
            y = jnp.tanh(x @ W + b)
            if W.shape[-1] == 2 * x.shape[-1]:
                y = y + jnp.concatenate([x, x], axis=-1)
            elif W.shape[-1] == x.shape[-1]:
                y = y + x
            x = y
        xyz = xyz + jnp.einsum("nsc,nsm->ncm", rr, x)
    xyz = xyz / NNEI
    return xyz.reshape(nf, nloc, 4, M).astype(jnp.bfloat16)


_pmapped = None


def _get_pmapped():
    global _pmapped
    if _pmapped is None:
        _pmapped = jax.pmap(_shard_fn, devices=jax.devices()[:NCORES])
    return _pmapped


def kernel(nlist, extended_coord, extended_atype, mean, stddev,
           w0, b0, w1, b1, w2, b2):
    nlist = np.asarray(nlist)
    coord = np.asarray(extended_coord, dtype=np.float32)  # [nf, nall, 3]
    atype = np.asarray(extended_atype)

    # frame offset baked into the indices; padding stays negative (int16 ok:
    # max index 2*8192-1 = 16383 < 32767)
    frame_off = (np.arange(NF, dtype=np.int64) * NALL)[:, None, None]
    nl_abs = np.where(nlist >= 0, nlist + frame_off, -1).astype(np.int16)

    nl_sh = np.ascontiguousarray(
        nl_abs.reshape(NF, NCORES, SHARD, NNEI).transpose(1, 0, 2, 3))
    centers_sh = np.ascontiguousarray(
        coord[:, :NLOC].reshape(NF, NCORES, SHARD, 3).transpose(1, 0, 2, 3))
    atype_sh = np.ascontiguousarray(
        atype[:, :NLOC].astype(np.int32).reshape(NF, NCORES, SHARD)
        .transpose(1, 0, 2))

    coord_all = coord.reshape(NF * NALL, 3)

    def rep(x):
        x = np.asarray(x, dtype=np.float32)
        return np.broadcast_to(x, (NCORES,) + x.shape)

    xyz = _get_pmapped()(
        nl_sh, rep(coord_all), centers_sh, atype_sh,
        rep(mean), rep(stddev),
        rep(w0), rep(b0), rep(w1), rep(b1), rep(w2), rep(b2),
    )  # [8, nf, shard, 4, M] bf16

    xyz = np.asarray(xyz).astype(np.float32)  # [8, 2, 512, 4, 100]
    M = xyz.shape[-1]
    xyz = xyz.transpose(1, 0, 2, 3, 4).reshape(NF * NLOC, 4, M)
    # res[n, m, a] = sum_c xyz[n, c, m] * xyz[n, c, a] for a < AXIS
    res = np.matmul(xyz.transpose(0, 2, 1), xyz[:, :, :AXIS])
    return np.ascontiguousarray(
        res.reshape(NF, NLOC, M * AXIS).astype(np.float32))
